# revision 1
# baseline (speedup 1.0000x reference)
"""Trainium2 Bass kernel for nn_NumDualDescriptorAB.

Reference computation:
    agg[b,w]   = mean(seq[b, w:w+8, :], axis=0)          (sliding window, Nw = S-7)
    y[b,w]     = agg[b,w] @ M.T
    Nk[w]      = Acoeff[:, w%L] * Bbasis[w%L, :]
    D          = mean((y - Nk)^2)

Algebraic decomposition (everything heavy becomes matmuls with tiny outputs):
    count = B*Nw*m
    t1 = sum_{b,w} agg MtM agg^T = <M^T M, G>_F   with G = sum agg^T agg   (m x m)
    t2 = sum_{b,w} y . Nk = sum_{b,s} seq[b,s] . P[s]    with P = W^T (Nk M)  (S x m)
    t3 = B * ||Nk||^2
    D  = (t1 - 2 t2 + t3) / count

Device (8 cores, data-parallel over batch; 4 batches/core) computes G and
X^T = sum_chunks P_chunk^T seq_chunk per core; the tiny combination happens
on host in float64.

The sliding-window aggregation itself runs on the TensorEngine via a banded
constant matrix W (lhsT), chunked 121 windows at a time so each chunk's
windows only need the chunk's own 128 rows.
"""

import numpy as np
import ml_dtypes

B, S, m, L, RANK = 32, 2048, 128, 64, 8
Nw = S - RANK + 1  # 2041
NCORES = 8
BPC = B // NCORES  # batches per core = 4
CH = 121  # windows per chunk (window w needs rows w..w+7, so 121+7=128 rows)
NCH = (Nw + CH - 1) // CH  # 17 chunks
TAILW = Nw - (NCH - 1) * CH  # 105 windows in the last chunk

BF16 = ml_dtypes.bfloat16

_NC_CACHE = {}


def _build_nc():
    import concourse.bacc as bacc
    import concourse.mybir as mybir
    import concourse.tile as tile

    bf = mybir.dt.bfloat16
    f32 = mybir.dt.float32

    nc = bacc.Bacc("TRN2", target_bir_lowering=False, debug=False,
                   num_devices=NCORES)

    seq_d = nc.dram_tensor("seq", [128, NCH * BPC * m], bf, kind="ExternalInput")
    w_d = nc.dram_tensor("wmat", [128, 2 * m], bf, kind="ExternalInput")
    p_d = nc.dram_tensor("pmat", [128, NCH * m], bf, kind="ExternalInput")
    out_d = nc.dram_tensor("out", [128, m + BPC * m], f32, kind="ExternalOutput")

    CW = BPC * m  # columns per chunk = 512
    GROUPS = [(0, 4), (4, 8), (8, 12), (12, NCH)]

    with tile.TileContext(nc) as tc:
        with (
            tc.tile_pool(name="const", bufs=1) as cpool,
            tc.tile_pool(name="agg", bufs=3) as apool,
            tc.tile_pool(name="psa", bufs=3, space="PSUM") as pspool,
            tc.tile_pool(name="psacc", bufs=1, space="PSUM") as accpool,
        ):
            s_w = cpool.tile([128, 2 * m], bf, tag="w")
            nc.sync.dma_start(out=s_w[:], in_=w_d[:])
            s_p = cpool.tile([128, NCH * m], bf, tag="p")
            nc.sync.dma_start(out=s_p[:], in_=p_d[:])

            seq_tiles = []
            for gi, (a, b) in enumerate(GROUPS):
                t = cpool.tile([128, (b - a) * CW], bf, tag=f"seq{gi}")
                nc.sync.dma_start(out=t[:], in_=seq_d[:, a * CW:b * CW])
                seq_tiles.append(t)

            G_ps = accpool.tile([128, m], f32, tag="G")
            X_ps = accpool.tile([128, BPC * m], f32, tag="X")

            for gi, (a, b) in enumerate(GROUPS):
                for c in range(a, b):
                    rhs = seq_tiles[gi][:, (c - a) * CW:(c - a + 1) * CW]
                    wsel = s_w[:, 0:m] if c < NCH - 1 else s_w[:, m:2 * m]

                    agg_ps = pspool.tile([128, CW], f32, tag="aggps")
                    nc.tensor.matmul(agg_ps[:], wsel, rhs, start=True, stop=True)
                    nc.tensor.matmul(
                        X_ps[:], s_p[:, c * m:(c + 1) * m], rhs,
                        start=(c == 0), stop=(c == NCH - 1),
                        skip_group_check=True,
                    )

                    aggb = apool.tile([128, CW], bf, tag="aggb")
                    if c % 2 == 0:
                        nc.vector.tensor_copy(aggb[:], agg_ps[:])
                    else:
                        nc.scalar.copy(aggb[:], agg_ps[:])

                    for j in range(BPC):
                        blk = aggb[:, j * m:(j + 1) * m]
                        nc.tensor.matmul(
                            G_ps[:], blk, blk,
                            start=(c == 0 and j == 0),
                            stop=(c == NCH - 1 and j == BPC - 1),
                            skip_group_check=True,
                        )

            s_out = cpool.tile([128, m + BPC * m], f32, tag="out")
            nc.vector.tensor_copy(s_out[:, 0:m], G_ps[:])
            nc.scalar.copy(s_out[:, m:m + BPC * m], X_ps[:])
            nc.sync.dma_start(out=out_d[:], in_=s_out[:])

    nc.compile()
    return nc


def get_nc():
    if "nc" not in _NC_CACHE:
        _NC_CACHE["nc"] = _build_nc()
    return _NC_CACHE["nc"]


def _chunk_rows():
    rows = CH * np.arange(NCH)[:, None] + np.arange(128)[None, :]  # [NCH, 128]
    valid = rows < S
    return rows, valid


def host_prep(seq_batch, M, Acoeff, Bbasis):
    """Build per-core device inputs + host-side exact terms."""
    rows, valid = _chunk_rows()
    rows_c = np.minimum(rows, S - 1)

    # seq image: per core [128, NCH, BPC, m] with seq_img[p, c, j] = seq[4k+j, 121c+p]
    g = seq_batch[:, rows_c, :].astype(BF16)  # [B, NCH, 128, m]
    g[:, ~valid, :] = 0
    imgs = np.ascontiguousarray(
        g.reshape(NCORES, BPC, NCH, 128, m).transpose(0, 3, 2, 1, 4)
    ).reshape(NCORES, 128, NCH * BPC * m)

    # banded window matrices (lhsT): out[w, n] = sum_k W[k, w] rhs[k, n]
    k = np.arange(128)[:, None]
    w = np.arange(128)[None, :]
    band = ((k - w >= 0) & (k - w < RANK)).astype(np.float32) / RANK
    wmain = band * (w < CH)
    wtail = band * (w < TAILW)
    wmat = np.concatenate([wmain, wtail], axis=1).astype(BF16)  # [128, 256]

    # Nk / Ntil / P in float64
    M64 = np.asarray(M, np.float64)
    kmod = np.arange(Nw) % L
    Nk = (np.asarray(Acoeff, np.float64).T[kmod]
          * np.asarray(Bbasis, np.float64)[kmod])  # [Nw, m]
    Ntil = Nk @ M64  # [Nw, m]
    csum = np.concatenate([np.zeros((1, m)), np.cumsum(Ntil, axis=0)])
    s = np.arange(S)
    lo = np.maximum(s - (RANK - 1), 0)
    hi = np.minimum(s, Nw - 1)
    P = (csum[hi + 1] - csum[lo]) / RANK  # [S, m]

    pr = P[rows_c].astype(np.float32)  # [NCH, 128, m]
    pvalid = valid & (np.arange(128) < CH)[None, :]
    pr[~pvalid] = 0
    pmat = np.ascontiguousarray(pr.transpose(1, 0, 2)).reshape(128, NCH * m).astype(BF16)

    t3 = B * float((Nk ** 2).sum())
    MtM = M64.T @ M64
    return imgs, wmat, pmat, MtM, t3


def combine(results, MtM, t3):
    """results: list of 8 arrays [128, 640] f32 -> scalar D."""
    G = np.zeros((m, m), np.float64)
    t2 = 0.0
    for r in results:
        r = np.asarray(r, np.float64)
        G += r[:, :m]
        for j in range(BPC):
            t2 += np.trace(r[:, m + j * m:m + (j + 1) * m])
    t1 = float((MtM * G).sum())
    D = (t1 - 2.0 * t2 + t3) / (B * Nw * m)
    return np.float32(D)


def kernel(seq_batch, M, Acoeff, Bbasis):
    from concourse.bass_utils import run_bass_kernel_spmd

    seq_batch = np.asarray(seq_batch, np.float32)
    imgs, wmat, pmat, MtM, t3 = host_prep(seq_batch, M, Acoeff, Bbasis)

    nc = get_nc()
    in_maps = [
        {"seq": imgs[c], "wmat": wmat, "pmat": pmat} for c in range(NCORES)
    ]
    res = run_bass_kernel_spmd(nc, in_maps, core_ids=list(range(NCORES)))
    outs = [res.results[c]["out"] for c in range(NCORES)]
    return combine(outs, MtM, t3)
